# revision 5
# baseline (speedup 1.0000x reference)
"""Multi-head attention forward on 8 Trainium2 NeuronCores (Bass/Tile).

Problem: nn_MultiHeadAttention — B=8, T=1024, C=768, H=12, D=64, fp32.

Sharding: data-parallel over batch — B=8 → one batch element per core; weights
broadcast. No collectives.

Per-core kernel (x^T pre-transposed on host to [C, T]):
  1. Q^T, K^T = (Wq|Wk)^T-contract:  psum[c_out 128, t 512] = sum_k Wq[k,co]^T...
     matmul(lhsT=W[:,k,co], rhs=xT[:,k,t]) accumulated over k — gives head-major
     transposed layout [D, T] per head, exactly what QK^T needs.
  2. V natural [T, C] via matmul(lhsT=xT[:,k,tchunk], rhs=Wv[:,k,:]), stored into
     V_aug [128, TS, H, D+1] with a ones column — the ones row of the AV product
     yields the softmax denominator for free.
  3. Per head h, i-chunk: S^T[j, i] = matmul(lhsT=KT_h[:, j128], rhs=QT_h[:, i512])
     (K=64); P = exp(S^T/8) via ScalarE (no max subtraction needed: logits ~N(0,1));
     Ytil psum[65, i512] += matmul(lhsT=V_aug[:, j, h, :], rhs=P) over j.
  4. y^T rows = Ytil[0:64] * reciprocal(Ytil[64]) (DVE recip + GpSimd partition
     broadcast), written into Y^T [C, T] layout.
  5. out[t, c] = matmul(lhsT=YT[:, k, t128], rhs=Wp[:, k, :]) + bp → DMA to y.

All matmul operands are float32r (TF32-like fast fp32 mode, ~1.5e-4 rel err).
"""
import numpy as np

B, T, C = 8, 1024, 768
H, D = 12, 64
P = 128
KS = C // P          # 6 contraction subtiles
TS = T // P          # 8 t subtiles
NI = T // 512        # 2 i-chunks of 512
N_CORES = 8

_RUNNER_CACHE = {}


def build_nc(reps: int = 1):
    import concourse.bacc as bacc
    import concourse.mybir as mybir
    import concourse.tile as tile
    from contextlib import ExitStack

    f32 = mybir.dt.float32
    f32r = mybir.dt.float32r
    AF = mybir.ActivationFunctionType
    ALU = mybir.AluOpType

    nc = bacc.Bacc(num_devices=N_CORES)

    xT_d = nc.dram_tensor("xT", [C, T], f32r, kind="ExternalInput")
    W_d = {w: nc.dram_tensor(f"W{w}", [C, C], f32r, kind="ExternalInput")
           for w in ("q", "k", "v", "p")}
    bqT_d = nc.dram_tensor("bqT", [P, KS], f32, kind="ExternalInput")
    bkT_d = nc.dram_tensor("bkT", [P, KS], f32, kind="ExternalInput")
    bvB_d = nc.dram_tensor("bvB", [P, C], f32, kind="ExternalInput")
    bpB_d = nc.dram_tensor("bpB", [P, C], f32, kind="ExternalInput")
    y_d = nc.dram_tensor("y", [T, C], f32, kind="ExternalOutput")

    with tile.TileContext(nc) as tc, ExitStack() as ctx:
        const = ctx.enter_context(tc.tile_pool(name="const", bufs=1))
        ppool = ctx.enter_context(tc.tile_pool(name="pt", bufs=3))
        npool = ctx.enter_context(tc.tile_pool(name="norm", bufs=2))
        opool = ctx.enter_context(tc.tile_pool(name="out", bufs=2))
        psQ = ctx.enter_context(tc.tile_pool(name="psQ", bufs=2, space="PSUM"))
        psS = ctx.enter_context(tc.tile_pool(name="psS", bufs=2, space="PSUM"))
        psY = ctx.enter_context(tc.tile_pool(name="psY", bufs=2, space="PSUM"))

        def body(_iv=None):
            # ---- loads ----
            xTr = const.tile([P, KS, T], f32r, tag="xT", name="xTr")
            nc.sync.dma_start(xTr[:], xT_d.rearrange("(ks p) t -> p ks t", p=P))
            Wr = {}
            for w in ("q", "k", "v"):
                Wr[w] = const.tile([P, KS, C], f32r, tag=f"W{w}", name=f"W{w}r")
                nc.sync.dma_start(Wr[w][:], W_d[w].rearrange("(ks p) c -> p ks c", p=P))
            bqT = const.tile([P, KS], f32, tag="bqT", name="bqT")
            nc.sync.dma_start(bqT[:], bqT_d[:, :])
            bkT = const.tile([P, KS], f32, tag="bkT", name="bkT")
            nc.sync.dma_start(bkT[:], bkT_d[:, :])
            bvB = const.tile([P, C], f32, tag="bvB", name="bvB")
            nc.sync.dma_start(bvB[:], bvB_d[:, :])
            bpB = const.tile([P, C], f32, tag="bpB", name="bpB")
            nc.sync.dma_start(bpB[:], bpB_d[:, :])
            ones1 = const.tile([P, 1], f32, tag="ones", name="ones1")
            nc.vector.memset(ones1[:], 1.0)

            # ---- Q^T, K^T projections ----
            QT = const.tile([P, KS, T], f32r, tag="QT", name="QT")
            KT = const.tile([P, KS, T], f32r, tag="KT", name="KT")
            for dst, w, bias in ((QT, "q", bqT), (KT, "k", bkT)):
                for co in range(KS):
                    ps = [psQ.tile([P, 512], f32, tag="psQ", name="psq") for _ in range(NI)]
                    for k in range(KS):
                        lhsT = Wr[w][:, k, co * P:(co + 1) * P]
                        for ti in range(NI):
                            nc.tensor.matmul(ps[ti][:], lhsT,
                                             xTr[:, k, ti * 512:(ti + 1) * 512],
                                             start=(k == 0), stop=(k == KS - 1))
                    for ti in range(NI):
                        nc.vector.tensor_tensor(
                            dst[:, co, ti * 512:(ti + 1) * 512], ps[ti][:],
                            bias[:, co:co + 1].to_broadcast([P, 512]), op=ALU.add)

            # Wp reuses Wq's SBUF slot (Wq dead after the Q^T projection)
            Wr["p"] = const.tile([P, KS, C], f32r, tag="Wq", name="Wpr")
            nc.sync.dma_start(Wr["p"][:], W_d["p"].rearrange("(ks p) c -> p ks c", p=P))

            # ---- V (natural layout) into V_aug with ones column ----
            V_aug = const.tile([P, TS, H, D + 1], f32r, tag="Vaug", name="Vaug")
            nc.vector.tensor_copy(V_aug[:, :, :, D:D + 1],
                                  ones1[:].to_broadcast([P, TS, H, 1]))
            for ts_ in range(TS):
                psv = [psQ.tile([P, 512], f32, tag="psQ", name="psq") for _ in range(2)]
                for k in range(KS):
                    lhsT = xTr[:, k, ts_ * P:(ts_ + 1) * P]
                    nc.tensor.matmul(psv[0][:], lhsT, Wr["v"][:, k, 0:512],
                                     start=(k == 0), stop=(k == KS - 1))
                    nc.tensor.matmul(psv[1][:, 0:256], lhsT, Wr["v"][:, k, 512:768],
                                     start=(k == 0), stop=(k == KS - 1))
                nc.vector.tensor_tensor(
                    V_aug[:, ts_, 0:8, 0:D],
                    psv[0][:].rearrange("p (h d) -> p h d", h=8),
                    bvB[:, 0:512].rearrange("p (h d) -> p h d", h=8), op=ALU.add)
                nc.vector.tensor_tensor(
                    V_aug[:, ts_, 8:12, 0:D],
                    psv[1][:, 0:256].rearrange("p (h d) -> p h d", h=4),
                    bvB[:, 512:768].rearrange("p (h d) -> p h d", h=4), op=ALU.add)

            # ---- attention + AV per head ----
            YT = const.tile([P, KS, T], f32r, tag="xT", name="YT")
            for h in range(H):
                po, hh = h // 2, h % 2
                b0 = 64 * hh
                psy = [psY.tile([P, 512], f32, tag="psY", name="psy") for _ in range(NI)]
                for j in range(TS):
                    pss = psS.tile([P, 1024], f32, tag="psS", name="pss")
                    lhsT = KT[b0:b0 + 64, po, j * P:(j + 1) * P]
                    for i in range(NI):
                        nc.tensor.matmul(pss[:, i * 512:(i + 1) * 512], lhsT,
                                         QT[b0:b0 + 64, po, i * 512:(i + 1) * 512],
                                         start=True, stop=True)
                    pt = ppool.tile([P, 1024], f32r, tag="pt", name="pt")
                    nc.scalar.activation(pt[:], pss[:], AF.Exp, scale=0.125)
                    for i in range(NI):
                        nc.tensor.matmul(psy[i][0:D + 1, :], V_aug[:, j, h, :],
                                         pt[:, i * 512:(i + 1) * 512],
                                         start=(j == 0), stop=(j == TS - 1))
                # normalize: y^T = Ytil[0:64] * recip(Ytil[64])
                for i in range(NI):
                    dd = npool.tile([1, 512], f32, tag="dd", name="dd")
                    nc.vector.tensor_copy(dd[0:1, :], psy[i][D:D + 1, :])
                    rr = npool.tile([1, 512], f32, tag="rr", name="rr")
                    nc.vector.reciprocal_approx_fast(rr[0:1, :], dd[0:1, :])
                    rb = npool.tile([D, 512], f32, tag="rb", name="rb")
                    nc.gpsimd.partition_broadcast(rb[:], rr[0:1, :])
                    nc.vector.tensor_tensor(
                        YT[b0:b0 + 64, po, i * 512:(i + 1) * 512],
                        psy[i][0:D, :], rb[:], op=ALU.mult)

            # ---- output projection ----
            for ts_ in range(TS):
                po_ = [psQ.tile([P, 512], f32, tag="psQ", name="psq") for _ in range(2)]
                for k in range(KS):
                    lhsT = YT[:, k, ts_ * P:(ts_ + 1) * P]
                    nc.tensor.matmul(po_[0][:], lhsT, Wr["p"][:, k, 0:512],
                                     start=(k == 0), stop=(k == KS - 1))
                    nc.tensor.matmul(po_[1][:, 0:256], lhsT, Wr["p"][:, k, 512:768],
                                     start=(k == 0), stop=(k == KS - 1))
                ot = opool.tile([P, C], f32, tag="ot", name="ot")
                nc.vector.tensor_tensor(ot[:, 0:512], po_[0][:], bpB[:, 0:512],
                                        op=ALU.add)
                nc.vector.tensor_tensor(ot[:, 512:768], po_[1][:, 0:256],
                                        bpB[:, 512:768], op=ALU.add)
                nc.sync.dma_start(y_d[ts_ * P:(ts_ + 1) * P, :], ot[:])

        if reps == 1:
            body()
        else:
            with tc.For_i(0, reps, 1):
                body()

    nc.compile()
    return nc


class _Runner:
    """Compile once, run many times on the 8 axon-tunneled cores via PJRT."""

    def __init__(self, nc, n_cores):
        import jax
        import concourse.mybir as mybir
        from jax.sharding import Mesh, PartitionSpec
        from jax.experimental.shard_map import shard_map
        from concourse.bass2jax import (
            _bass_exec_p, install_neuronx_cc_hook, partition_id_tensor)

        install_neuronx_cc_hook()
        self.jax = jax
        self.n_cores = n_cores
        partition_name = nc.partition_id_tensor.name if nc.partition_id_tensor else None
        in_names, out_names, out_avals, zero_outs = [], [], [], []
        for alloc in nc.m.functions[0].allocations:
            if not isinstance(alloc, mybir.MemoryLocationSet):
                continue
            name = alloc.memorylocations[0].name
            if alloc.kind == "ExternalInput":
                if name != partition_name:
                    in_names.append(name)
            elif alloc.kind == "ExternalOutput":
                shape = tuple(alloc.tensor_shape)
                dtype = mybir.dt.np(alloc.dtype)
                out_names.append(name)
                out_avals.append(jax.core.ShapedArray(shape, dtype))
                zero_outs.append(np.zeros(shape, dtype))
        self.in_names, self.out_names = in_names, out_names
        self.zero_outs = zero_outs
        all_in = list(in_names) + list(out_names)
        if partition_name is not None:
            all_in.append(partition_name)

        def _body(*args):
            operands = list(args)
            if partition_name is not None:
                operands.append(partition_id_tensor())
            return tuple(_bass_exec_p.bind(
                *operands, out_avals=tuple(out_avals), in_names=tuple(all_in),
                out_names=tuple(out_names), lowering_input_output_aliases=(),
                sim_require_finite=True, sim_require_nnan=True, nc=nc))

        devices = jax.devices()[:n_cores]
        self.mesh = Mesh(np.asarray(devices), ("core",))
        spec = PartitionSpec("core")
        self.fn = jax.jit(
            shard_map(_body, mesh=self.mesh,
                      in_specs=(spec,) * (len(in_names) + len(out_names)),
                      out_specs=(spec,) * len(out_names), check_rep=False),
            keep_unused=True)

    def stage(self, in_maps):
        import jax
        from jax.sharding import PartitionSpec
        concat = [
            np.concatenate([np.asarray(in_maps[c][n]) for c in range(self.n_cores)], axis=0)
            for n in self.in_names
        ] + [np.concatenate([z] * self.n_cores, axis=0) for z in self.zero_outs]
        sharding = jax.sharding.NamedSharding(self.mesh, PartitionSpec("core"))
        return [jax.device_put(a, sharding) for a in concat]

    def run(self, staged):
        outs = self.fn(*staged)
        self.jax.block_until_ready(outs)
        return outs

    def run_to_maps(self, staged):
        outs = self.run(staged)
        res = []
        for c in range(self.n_cores):
            m = {}
            for i, n in enumerate(self.out_names):
                g = np.asarray(outs[i])
                per = g.shape[0] // self.n_cores
                m[n] = g[c * per:(c + 1) * per]
            res.append(m)
        return res


def get_runner(reps: int = 1):
    key = reps
    if key not in _RUNNER_CACHE:
        nc = build_nc(reps)
        _RUNNER_CACHE[key] = _Runner(nc, N_CORES)
    return _RUNNER_CACHE[key]


def make_in_maps(x, Wq, bq, Wk, bk, Wv, bv, Wp, bp):
    x = np.asarray(x, dtype=np.float32)
    weights = {
        "Wq": np.asarray(Wq, np.float32), "Wk": np.asarray(Wk, np.float32),
        "Wv": np.asarray(Wv, np.float32), "Wp": np.asarray(Wp, np.float32),
    }
    bqT = np.ascontiguousarray(np.asarray(bq, np.float32).reshape(KS, P).T)
    bkT = np.ascontiguousarray(np.asarray(bk, np.float32).reshape(KS, P).T)
    bvB = np.ascontiguousarray(np.broadcast_to(np.asarray(bv, np.float32), (P, C)))
    bpB = np.ascontiguousarray(np.broadcast_to(np.asarray(bp, np.float32), (P, C)))
    in_maps = []
    for b in range(B):
        in_maps.append({
            "xT": np.ascontiguousarray(x[b].T),
            "Wq": weights["Wq"], "Wk": weights["Wk"],
            "Wv": weights["Wv"], "Wp": weights["Wp"],
            "bqT": bqT, "bkT": bkT, "bvB": bvB, "bpB": bpB,
        })
    return in_maps


def kernel(x, Wq, bq, Wk, bk, Wv, bv, Wp, bp):
    runner = get_runner(reps=1)
    in_maps = make_in_maps(x, Wq, bq, Wk, bk, Wv, bv, Wp, bp)
    staged = runner.stage(in_maps)
    res = runner.run_to_maps(staged)
    return np.stack([res[b]["y"] for b in range(B)], axis=0)


# revision 7
# speedup vs baseline: 1.2956x; 1.2956x over previous
"""Multi-head attention forward on 8 Trainium2 NeuronCores (Bass/Tile).

Problem: nn_MultiHeadAttention — B=8, T=1024, C=768, H=12, D=64, fp32.

Sharding: data-parallel over batch — B=8 → one batch element per core; weights
broadcast. No collectives.

Per-core kernel (x^T pre-transposed on host to [C, T]):
  1. Q^T, K^T = (Wq|Wk)^T-contract:  psum[c_out 128, t 512] = sum_k Wq[k,co]^T...
     matmul(lhsT=W[:,k,co], rhs=xT[:,k,t]) accumulated over k — gives head-major
     transposed layout [D, T] per head, exactly what QK^T needs.
  2. V natural [T, C] via matmul(lhsT=xT[:,k,tchunk], rhs=Wv[:,k,:]), stored into
     V_aug [128, TS, H, D+1] with a ones column — the ones row of the AV product
     yields the softmax denominator for free.
  3. Per head h, i-chunk: S^T[j, i] = matmul(lhsT=KT_h[:, j128], rhs=QT_h[:, i512])
     (K=64); P = exp(S^T/8) via ScalarE (no max subtraction needed: logits ~N(0,1));
     Ytil psum[65, i512] += matmul(lhsT=V_aug[:, j, h, :], rhs=P) over j.
  4. y^T rows = Ytil[0:64] * reciprocal(Ytil[64]) (DVE recip + GpSimd partition
     broadcast), written into Y^T [C, T] layout.
  5. out[t, c] = matmul(lhsT=YT[:, k, t128], rhs=Wp[:, k, :]) + bp → DMA to y.

All matmul operands are float32r (TF32-like fast fp32 mode, ~1.5e-4 rel err).
"""
import numpy as np

B, T, C = 8, 1024, 768
H, D = 12, 64
P = 128
KS = C // P          # 6 contraction subtiles
TS = T // P          # 8 t subtiles
NI = T // 512        # 2 i-chunks of 512
N_CORES = 8

_RUNNER_CACHE = {}


def build_nc(reps: int = 1):
    import concourse.bacc as bacc
    import concourse.mybir as mybir
    import concourse.tile as tile
    from contextlib import ExitStack

    f32 = mybir.dt.float32
    f32r = mybir.dt.float32r
    AF = mybir.ActivationFunctionType
    ALU = mybir.AluOpType

    nc = bacc.Bacc(num_devices=N_CORES)

    xT_d = nc.dram_tensor("xT", [C, T], f32r, kind="ExternalInput")
    W_d = {w: nc.dram_tensor(f"W{w}", [C, C], f32r, kind="ExternalInput")
           for w in ("q", "k", "v", "p")}
    bqT_d = nc.dram_tensor("bqT", [P, KS], f32, kind="ExternalInput")
    bkT_d = nc.dram_tensor("bkT", [P, KS], f32, kind="ExternalInput")
    bvB_d = nc.dram_tensor("bvB", [P, C], f32, kind="ExternalInput")
    bpB_d = nc.dram_tensor("bpB", [P, C], f32, kind="ExternalInput")
    y_d = nc.dram_tensor("y", [T, C], f32, kind="ExternalOutput")

    with tile.TileContext(nc) as tc, ExitStack() as ctx:
        const = ctx.enter_context(tc.tile_pool(name="const", bufs=1))
        ppool = ctx.enter_context(tc.tile_pool(name="pt", bufs=3))
        npool = ctx.enter_context(tc.tile_pool(name="norm", bufs=2))
        opool = ctx.enter_context(tc.tile_pool(name="out", bufs=2))
        psQ = ctx.enter_context(tc.tile_pool(name="psQ", bufs=2, space="PSUM"))
        psS = ctx.enter_context(tc.tile_pool(name="psS", bufs=2, space="PSUM"))
        psY = ctx.enter_context(tc.tile_pool(name="psY", bufs=2, space="PSUM"))

        def body(_iv=None):
            # ---- loads ----
            xTr = const.tile([P, KS, T], f32r, tag="xT", name="xTr")
            nc.sync.dma_start(xTr[:], xT_d.rearrange("(ks p) t -> p ks t", p=P))
            Wr = {}
            for w in ("q", "k", "v"):
                Wr[w] = const.tile([P, KS, C], f32r, tag=f"W{w}", name=f"W{w}r")
                nc.sync.dma_start(Wr[w][:], W_d[w].rearrange("(ks p) c -> p ks c", p=P))
            bqT = const.tile([P, KS], f32, tag="bqT", name="bqT")
            nc.sync.dma_start(bqT[:], bqT_d[:, :])
            bkT = const.tile([P, KS], f32, tag="bkT", name="bkT")
            nc.sync.dma_start(bkT[:], bkT_d[:, :])
            bvB = const.tile([P, C], f32, tag="bvB", name="bvB")
            nc.sync.dma_start(bvB[:], bvB_d[:, :])
            bpB = const.tile([P, C], f32, tag="bpB", name="bpB")
            nc.sync.dma_start(bpB[:], bpB_d[:, :])
            ones1 = const.tile([P, 1], f32, tag="ones", name="ones1")
            nc.vector.memset(ones1[:], 1.0)

            # ---- V (natural layout) into V_aug with ones column ----
            V_aug = const.tile([P, TS, H, D + 1], f32r, tag="Vaug", name="Vaug")
            nc.vector.tensor_copy(V_aug[:, :, :, D:D + 1],
                                  ones1[:].to_broadcast([P, TS, H, 1]))
            for ts_ in range(TS):
                psv = [psQ.tile([P, 512], f32, tag="psQ", name="psq") for _ in range(2)]
                for k in range(KS):
                    lhsT = xTr[:, k, ts_ * P:(ts_ + 1) * P]
                    nc.tensor.matmul(psv[0][:], lhsT, Wr["v"][:, k, 0:512],
                                     start=(k == 0), stop=(k == KS - 1))
                    nc.tensor.matmul(psv[1][:, 0:256], lhsT, Wr["v"][:, k, 512:768],
                                     start=(k == 0), stop=(k == KS - 1))
                nc.vector.tensor_tensor(
                    V_aug[:, ts_, 0:8, 0:D],
                    psv[0][:].rearrange("p (h d) -> p h d", h=8),
                    bvB[:, 0:512].rearrange("p (h d) -> p h d", h=8), op=ALU.add)
                nc.vector.tensor_tensor(
                    V_aug[:, ts_, 8:12, 0:D],
                    psv[1][:, 0:256].rearrange("p (h d) -> p h d", h=4),
                    bvB[:, 512:768].rearrange("p (h d) -> p h d", h=4), op=ALU.add)

            # ---- per-pair: Q^T/K^T projection for po=p, then attention ----
            # Per-po QT/KT tiles keep dependencies region-exact so pair p+1's
            # projection matmuls overlap pair p's (ScalarE-bound) attention.
            YT = const.tile([P, KS, T], f32r, tag="YTx", name="YT")
            QTs, KTs = {}, {}
            for h in range(H):
                po, hh = h // 2, h % 2
                b0 = 64 * hh
                if hh == 0:
                    # project Q^T, K^T for this head pair (c_out chunk po)
                    QTs[po] = const.tile([P, T], f32r, tag=f"QT{po % 2}", name="QTp")
                    KTs[po] = const.tile([P, T], f32r, tag=f"KT{po % 2}", name="KTp")
                    for dst, w, bias in ((QTs[po], "q", bqT), (KTs[po], "k", bkT)):
                        ps = [psQ.tile([P, 512], f32, tag="psQ", name="psq")
                              for _ in range(NI)]
                        for k in range(KS):
                            lhsT = Wr[w][:, k, po * P:(po + 1) * P]
                            for ti in range(NI):
                                nc.tensor.matmul(ps[ti][:], lhsT,
                                                 xTr[:, k, ti * 512:(ti + 1) * 512],
                                                 start=(k == 0), stop=(k == KS - 1))
                        for ti in range(NI):
                            nc.vector.tensor_tensor(
                                dst[:, ti * 512:(ti + 1) * 512], ps[ti][:],
                                bias[:, po:po + 1].to_broadcast([P, 512]), op=ALU.add)
                QT, KT = QTs[po], KTs[po]
                psy = [psY.tile([P, 512], f32, tag="psY", name="psy") for _ in range(NI)]
                for j in range(TS):
                    pss = psS.tile([P, 1024], f32, tag="psS", name="pss")
                    lhsT = KT[b0:b0 + 64, j * P:(j + 1) * P]
                    for i in range(NI):
                        nc.tensor.matmul(pss[:, i * 512:(i + 1) * 512], lhsT,
                                         QT[b0:b0 + 64, i * 512:(i + 1) * 512],
                                         start=True, stop=True)
                    pt = ppool.tile([P, 1024], f32r, tag="pt", name="pt")
                    nc.scalar.activation(pt[:], pss[:], AF.Exp, scale=0.125)
                    for i in range(NI):
                        nc.tensor.matmul(psy[i][0:D + 1, :], V_aug[:, j, h, :],
                                         pt[:, i * 512:(i + 1) * 512],
                                         start=(j == 0), stop=(j == TS - 1))
                # normalize: y^T = Ytil[0:64] * recip(Ytil[64])
                for i in range(NI):
                    dd = npool.tile([1, 512], f32, tag="dd", name="dd")
                    nc.vector.tensor_copy(dd[0:1, :], psy[i][D:D + 1, :])
                    rr = npool.tile([1, 512], f32, tag="rr", name="rr")
                    nc.vector.reciprocal_approx_fast(rr[0:1, :], dd[0:1, :])
                    rb = npool.tile([D, 512], f32, tag="rb", name="rb")
                    nc.gpsimd.partition_broadcast(rb[:], rr[0:1, :])
                    nc.vector.tensor_tensor(
                        YT[b0:b0 + 64, po, i * 512:(i + 1) * 512],
                        psy[i][0:D, :], rb[:], op=ALU.mult)

            # Wp load (deferred; needed only by the output projection)
            Wr["p"] = const.tile([P, KS, C], f32r, tag="Wv", name="Wpr")
            nc.sync.dma_start(Wr["p"][:], W_d["p"].rearrange("(ks p) c -> p ks c", p=P))

            # ---- output projection ----
            for ts_ in range(TS):
                po_ = [psQ.tile([P, 512], f32, tag="psQ", name="psq") for _ in range(2)]
                for k in range(KS):
                    lhsT = YT[:, k, ts_ * P:(ts_ + 1) * P]
                    nc.tensor.matmul(po_[0][:], lhsT, Wr["p"][:, k, 0:512],
                                     start=(k == 0), stop=(k == KS - 1))
                    nc.tensor.matmul(po_[1][:, 0:256], lhsT, Wr["p"][:, k, 512:768],
                                     start=(k == 0), stop=(k == KS - 1))
                ot = opool.tile([P, C], f32, tag="ot", name="ot")
                nc.vector.tensor_tensor(ot[:, 0:512], po_[0][:], bpB[:, 0:512],
                                        op=ALU.add)
                nc.vector.tensor_tensor(ot[:, 512:768], po_[1][:, 0:256],
                                        bpB[:, 512:768], op=ALU.add)
                nc.sync.dma_start(y_d[ts_ * P:(ts_ + 1) * P, :], ot[:])

        if reps == 1:
            body()
        else:
            with tc.For_i(0, reps, 1):
                body()

    nc.compile()
    return nc


class _Runner:
    """Compile once, run many times on the 8 axon-tunneled cores via PJRT."""

    def __init__(self, nc, n_cores):
        import jax
        import concourse.mybir as mybir
        from jax.sharding import Mesh, PartitionSpec
        from jax.experimental.shard_map import shard_map
        from concourse.bass2jax import (
            _bass_exec_p, install_neuronx_cc_hook, partition_id_tensor)

        install_neuronx_cc_hook()
        self.jax = jax
        self.n_cores = n_cores
        partition_name = nc.partition_id_tensor.name if nc.partition_id_tensor else None
        in_names, out_names, out_avals, zero_outs = [], [], [], []
        for alloc in nc.m.functions[0].allocations:
            if not isinstance(alloc, mybir.MemoryLocationSet):
                continue
            name = alloc.memorylocations[0].name
            if alloc.kind == "ExternalInput":
                if name != partition_name:
                    in_names.append(name)
            elif alloc.kind == "ExternalOutput":
                shape = tuple(alloc.tensor_shape)
                dtype = mybir.dt.np(alloc.dtype)
                out_names.append(name)
                out_avals.append(jax.core.ShapedArray(shape, dtype))
                zero_outs.append(np.zeros(shape, dtype))
        self.in_names, self.out_names = in_names, out_names
        self.zero_outs = zero_outs
        all_in = list(in_names) + list(out_names)
        if partition_name is not None:
            all_in.append(partition_name)

        def _body(*args):
            operands = list(args)
            if partition_name is not None:
                operands.append(partition_id_tensor())
            return tuple(_bass_exec_p.bind(
                *operands, out_avals=tuple(out_avals), in_names=tuple(all_in),
                out_names=tuple(out_names), lowering_input_output_aliases=(),
                sim_require_finite=True, sim_require_nnan=True, nc=nc))

        devices = jax.devices()[:n_cores]
        self.mesh = Mesh(np.asarray(devices), ("core",))
        spec = PartitionSpec("core")
        self.fn = jax.jit(
            shard_map(_body, mesh=self.mesh,
                      in_specs=(spec,) * (len(in_names) + len(out_names)),
                      out_specs=(spec,) * len(out_names), check_rep=False),
            keep_unused=True)

    def stage(self, in_maps):
        import jax
        from jax.sharding import PartitionSpec
        concat = [
            np.concatenate([np.asarray(in_maps[c][n]) for c in range(self.n_cores)], axis=0)
            for n in self.in_names
        ] + [np.concatenate([z] * self.n_cores, axis=0) for z in self.zero_outs]
        sharding = jax.sharding.NamedSharding(self.mesh, PartitionSpec("core"))
        return [jax.device_put(a, sharding) for a in concat]

    def run(self, staged):
        outs = self.fn(*staged)
        self.jax.block_until_ready(outs)
        return outs

    def run_to_maps(self, staged):
        outs = self.run(staged)
        res = []
        for c in range(self.n_cores):
            m = {}
            for i, n in enumerate(self.out_names):
                g = np.asarray(outs[i])
                per = g.shape[0] // self.n_cores
                m[n] = g[c * per:(c + 1) * per]
            res.append(m)
        return res


def get_runner(reps: int = 1):
    key = reps
    if key not in _RUNNER_CACHE:
        nc = build_nc(reps)
        _RUNNER_CACHE[key] = _Runner(nc, N_CORES)
    return _RUNNER_CACHE[key]


def make_in_maps(x, Wq, bq, Wk, bk, Wv, bv, Wp, bp):
    x = np.asarray(x, dtype=np.float32)
    weights = {
        "Wq": np.asarray(Wq, np.float32), "Wk": np.asarray(Wk, np.float32),
        "Wv": np.asarray(Wv, np.float32), "Wp": np.asarray(Wp, np.float32),
    }
    bqT = np.ascontiguousarray(np.asarray(bq, np.float32).reshape(KS, P).T)
    bkT = np.ascontiguousarray(np.asarray(bk, np.float32).reshape(KS, P).T)
    bvB = np.ascontiguousarray(np.broadcast_to(np.asarray(bv, np.float32), (P, C)))
    bpB = np.ascontiguousarray(np.broadcast_to(np.asarray(bp, np.float32), (P, C)))
    in_maps = []
    for b in range(B):
        in_maps.append({
            "xT": np.ascontiguousarray(x[b].T),
            "Wq": weights["Wq"], "Wk": weights["Wk"],
            "Wv": weights["Wv"], "Wp": weights["Wp"],
            "bqT": bqT, "bkT": bkT, "bvB": bvB, "bpB": bpB,
        })
    return in_maps


def kernel(x, Wq, bq, Wk, bk, Wv, bv, Wp, bp):
    runner = get_runner(reps=1)
    in_maps = make_in_maps(x, Wq, bq, Wk, bk, Wv, bv, Wp, bp)
    staged = runner.stage(in_maps)
    res = runner.run_to_maps(staged)
    return np.stack([res[b]["y"] for b in range(B)], axis=0)
